# revision 36
# baseline (speedup 1.0000x reference)
"""Trainium2 Bass kernel for the SG-visibility sampling network (v2).

Math notes (exploited structure):
  - U,V are orthogonal to the unit lobe axis l, so dot(sample_dir, l) == cos(r_phi)
    exactly.  Hence the SG weight w = exp(sharp*(cos_phi-1)) is a per-lobe
    constant and sum_s(vis*w)/(sum_s w + TINY) = scale_l * sum_s vis with
    scale_l = w/(S*w + TINY), precomputed on host.
  - pre-activation of the hidden layer decomposes as
        pre_h[n,l,s,h] = P_n[h] - C_l[h] - ct[n,l,s]*A_l[h] - st[n,l,s]*B_l[h]
    with P_n = p_n @ W1[:3] + b1,  A_l = sp_l*(U_l@Wd),  B_l = sp_l*(V_l@Wd),
    C_l = cp_l*(l_l@Wd),  Wd = root_rot @ W1[3:].
  - hemisphere mask: cos_term = ct*a_nl + st*b_nl + c_nl with
    a = normals@(sp*U)_l, b = normals@(sp*V)_l, c = normals@(cp*l)_l.
  - sigmoid(z) = 0.5*tanh(z/2) + 0.5, so vis*msk = 0.5*(tanh+1)*msk and the
    weighted sum runs as ONE accumulating matmul per (chunk, half) with
    scale_l/2 stationary.  Using Tanh instead of Sigmoid keeps every
    activation (Sin/Tanh/Relu/Copy) in ONE ACT table set (silu_and_others)
    -- zero LoadActFuncSet switches in steady state (the table-map cache is
    narrowed in _build_program so the greedy placement pass must pick it).

Device schedule (per core, data-parallel over N):
  - mask path in full fp32 (sign-exact): per-lobe a/b/c dots as 6 tiny fp32
    matmuls (TINY folded in via an ones-row so the compare is a single
    tensor_tensor is_gt), s-duplicated per chunk by one broadcast DMA each;
    mask trig = 2 fp32 Sin per chunk; q1/q2/q3/cmp + (tanh+1)*msk on DVE,
    issued AFTER each half's relu-drains so the DVE FIFO never blocks them.
  - hidden path in bf16: theta is HOST-PREBAKED into duplicated block layout
    (ct-rows | st-rows | point-rows per {7,7,2} sub-chunk) so one in-place Sin
    per sub-chunk tile produces the moving operand directly; per-lobe hidden
    matmul (bf16, K=116/36) runs LOOK=3 lobes ahead of the z matmuls (relu
    drain latency ~0.9us vs 426ns PE work per lobe), relu-drains alternate
    ACT/DVE (Pool cannot touch PSUM and walrus rejects ALU ops on Pool),
    K=128 bf16 matmul against block-diag W2 -> z in PSUM (double-buffered),
    one tanh per half, and a deferred scale matmul into the [L, n] output
    PSUM (issued mid-way through the NEXT half's z stream to dodge PE
    FIFO head-of-line blocking).
"""

import numpy as np

N, L, S, H = 8192, 128, 8, 16
NCORES = 8
NC = N // NCORES          # rays per core
LPC = 16                  # lobes per chunk
CHUNKS = L // LPC
TINY = 1e-6
HF = NC // 2              # moving-operand free-dim limit (512)

# inp (f32) row layout
R_RT = 0                  # rows 0..1023: r_theta in [l*S+s, n] layout
R_CB = L * S              # rows 1024..1151: cb [128, 8] in cols 0..7
INP_ROWS = R_CB + 128

# big (bf16, 1D): per chunk ONE [128, CB_COLS] "chunkbuf" holding the three
# theta block tiles AND the wcst stationary block side by side, so each
# partition row is a 10KB contiguous run -- DMA descriptor efficiency is
# ~4x better than per-tile 2KB rows (measured 28 -> ~113 GB/s per queue).
# cols: [blk0 | blk1 | blk2 | wcst] = [1024 | 1024 | 1024 | 2048]
SUBS = ((0, 7), (7, 7), (14, 2))   # (first lobe-in-chunk, n lobes) per sub-tile
CB_COLS = 3 * NC + LPC * 128       # 5120
O_BLK = 0
O_WSIG = O_BLK + CHUNKS * 128 * CB_COLS
O_WSUM = O_WSIG + 128 * 512
BIG_ELEMS = O_WSUM + 128 * CHUNKS * 128

_PROG = None


def _build_program():
    import concourse.bass as bass
    import concourse.bacc as bacc
    import concourse.mybir as mybir
    import concourse.tile as tile

    f32 = mybir.dt.float32
    bf16 = mybir.dt.bfloat16
    AF = mybir.ActivationFunctionType
    ALU = mybir.AluOpType
    PI4 = float(np.pi / 4.0)

    nc = bacc.Bacc("TRN2", target_bir_lowering=False, debug=False,
                   num_devices=NCORES)

    # The act-table-load pass greedily picks the FIRST table set containing
    # each activation's func, which alternates trig_and_small <-> exp_and_others
    # for Sin/Tanh (38 reloads, ~50us serialized on ACT).  silu_and_others
    # genuinely contains Sin+Tanh+Relu together; constrain the (cached) table
    # map so the pass can only pick it for Sin/Tanh.  Set ids are positional,
    # so entries are mutated in place -- never reordered.
    from concourse.hw_specs import get_activation_tables
    tabs = get_activation_tables(nc.m.arch)
    assert {AF.Sin, AF.Tanh, AF.Relu} <= tabs["silu_and_others"]
    for name, funcs in tabs.items():
        if name != "silu_and_others":
            funcs.discard(AF.Sin)
            funcs.discard(AF.Tanh)

    inp = nc.declare_dram_parameter("inp", [INP_ROWS, NC], f32, isOutput=False)
    big = nc.declare_dram_parameter("big", [BIG_ELEMS], bf16, isOutput=False)
    # host-prebaked s-duplicated hemisphere-mask dots, [128,(a|b|c')] per chunk
    abc = nc.declare_dram_parameter("abc", [CHUNKS * 128, 3 * NC], f32,
                                    isOutput=False)
    out = nc.declare_dram_parameter("out", [L, NC], f32, isOutput=True)

    def bslice(off, p, c):
        return big[off:off + p * c].rearrange("(p c) -> p c", p=p, c=c)

    # relu-drain engine per lobe-in-chunk (Pool/gpsimd cannot read PSUM, so
    # drains alternate ACT / DVE; Pool owns the SBUF-only mask chain)
    DRAIN = "ADADADADADADADAD"
    assert len(DRAIN) == LPC

    with tile.TileContext(nc) as tc:
        with (
            tc.tile_pool(name="const", bufs=1) as cpool,
            tc.tile_pool(name="io", bufs=3) as io,
            tc.tile_pool(name="trig", bufs=2) as trig,
            tc.tile_pool(name="abc", bufs=2) as abcp,
            tc.tile_pool(name="work", bufs=3) as work,
            tc.tile_pool(name="hrp", bufs=6) as hrp,
            tc.tile_pool(name="ps", bufs=5, space=bass.MemorySpace.PSUM) as ps,
            tc.tile_pool(name="zps", bufs=1, space=bass.MemorySpace.PSUM) as zps,
            tc.tile_pool(name="ops", bufs=1, space=bass.MemorySpace.PSUM) as opsp,
        ):
            # trigger the silu_and_others ACT table load during the startup
            # DMA window instead of right before the first real Sin
            warm = cpool.tile([1, 1], f32)
            nc.gpsimd.memset(warm[:], 0.0)
            nc.scalar.activation(warm[:], warm[:], AF.Sin)

            cb_t = cpool.tile([128, 8], f32)
            nc.sync.dma_start(cb_t[:], inp[R_CB:R_CB + 128, 0:8])

            out_ps = opsp.tile([128, NC], f32)
            pending_sum = None

            # One-chunk-ahead software pipeline: chunk C+1's DMAs AND all five
            # Sins (block trig + mask trig) issue at the TAIL of chunk C, so
            # chunk C's relu-drains find a conflict-free ACT FIFO and the mask
            # chain never waits on trig.
            def issue_loads(C):
                t = {}
                # one chunkbuf DMA per chunk (10KB descriptors); chunk 0
                # splits it so the blk0 Sin + first hidden matmuls can start
                # before the wcst tail lands.
                cbuf = io.tile([128, CB_COLS], bf16, tag="cbuf")
                src = big[O_BLK + C * 128 * CB_COLS:
                          O_BLK + (C + 1) * 128 * CB_COLS]
                src = src.rearrange("(p c) -> p c", p=128, c=CB_COLS)
                if C == 0:
                    # blk0 + leading wcst lobes first: gates the very first
                    # Sin + hidden matmuls (range-based dep tracking)
                    nc.sync.dma_start(cbuf[:, 0:NC], src[:, 0:NC])
                    nc.sync.dma_start(cbuf[:, 3 * NC:4 * NC], src[:, 3 * NC:4 * NC])
                    nc.sync.dma_start(cbuf[:, NC:3 * NC], src[:, NC:3 * NC])
                    nc.sync.dma_start(cbuf[:, 4 * NC:], src[:, 4 * NC:])
                else:
                    nc.sync.dma_start(cbuf[:], src)
                r_m = io.tile([128, NC], f32, tag="rm")
                nc.sync.dma_start(r_m[:], inp[C * 128:(C + 1) * 128, :])
                # host-prebaked s-dup'd a|b|c' rows, straight from HBM on the
                # scalar HWDGE queue -- issued while ACT idles waiting for the
                # first relu-drain, so neither the sync queue nor the ACT
                # compute stream pays for it.
                abc_C = abcp.tile([128, 3 * NC], f32, tag="abcC")
                nc.scalar.dma_start(abc_C[:], abc[C * 128:(C + 1) * 128, :])
                t["r_m"], t["cbuf"] = r_m, cbuf
                t["abc"] = abc_C
                return t

            def issue_block_sins(t):
                cbuf = t["cbuf"]
                nc.scalar.activation(cbuf[0:112, 0:NC], cbuf[0:112, 0:NC],
                                     AF.Sin, bias=cb_t[0:112, 5:6], scale=PI4)
                nc.scalar.activation(cbuf[0:112, NC:2 * NC],
                                     cbuf[0:112, NC:2 * NC],
                                     AF.Sin, bias=cb_t[0:112, 5:6], scale=PI4)
                nc.scalar.activation(cbuf[0:32, 2 * NC:2 * NC + NC],
                                     cbuf[0:32, 2 * NC:2 * NC + NC],
                                     AF.Sin, bias=cb_t[0:32, 6:7], scale=PI4)

            def issue_mask_sins(t):
                ct_m = trig.tile([128, NC], f32, tag="ct")
                st_m = trig.tile([128, NC], f32, tag="st")
                nc.scalar.activation(ct_m[:], t["r_m"][:], AF.Sin,
                                     bias=cb_t[:, 0:1], scale=PI4)
                nc.scalar.activation(st_m[:], t["r_m"][:], AF.Sin,
                                     bias=cb_t[:, 1:2], scale=PI4)
                t["ct"], t["st"] = ct_m, st_m

            # wsig gates the very first zmm (~3us in): small, ahead of the
            # chunk-0 loads on sync.  wsum (first sum ~14us in) rides the
            # scalar queue ahead of the bulky abc block.
            wsig_t = cpool.tile([128, 512], bf16)
            nc.sync.dma_start(wsig_t[:], bslice(O_WSIG, 128, 512))
            wsum_t = cpool.tile([128, CHUNKS * 128], bf16)
            nc.scalar.dma_start(wsum_t[:], bslice(O_WSUM, 128, CHUNKS * 128))

            cur = issue_loads(0)
            issue_block_sins(cur)
            issue_mask_sins(cur)

            for C in range(CHUNKS):
                nxt = issue_loads(C + 1) if C + 1 < CHUNKS else None
                cbuf = cur["cbuf"]
                abc_C = cur["abc"]
                ct_m, st_m = cur["ct"], cur["st"]
                last = C == CHUNKS - 1

                for hf in range(2):
                    fs = hf * HF
                    if hf == 1 and nxt is not None:
                        # chunk C+1's hidden matmuls gate on these; issuing
                        # mid-chunk keeps the ACT FIFO clear at the boundary
                        issue_block_sins(nxt)
                    zt = zps.tile([128, HF], f32, tag="zt")
                    # hidden matmuls run LOOK lobes ahead of the z matmuls so
                    # the PE FIFO never parks behind an in-flight relu-drain
                    # (drain latency ~0.9us vs 426ns of PE work per lobe).
                    LOOK = 4
                    hrs = [None] * LPC
                    # at hf1 the ACT queue starts with 3 block Sins; give the
                    # first drains to DVE there so zmm never waits on them
                    drain_eng = DRAIN if hf == 0 else DRAIN[::-1]

                    def hidden(j16):
                        k = min(j16 // 7, 2)
                        kv = 116 if k < 2 else 36
                        ph = ps.tile([128, HF], f32, tag="ph")
                        wc = 3 * NC + j16 * 128
                        nc.tensor.matmul(ph[:],
                                         cbuf[0:kv, wc:wc + 128],
                                         cbuf[0:kv, k * NC + fs:k * NC + fs + HF],
                                         start=True, stop=True)
                        hr = hrp.tile([128, HF], bf16, tag="hr")
                        if drain_eng[j16] == "A":
                            nc.scalar.activation(hr[:], ph[:], AF.Relu,
                                                 bias=cb_t[:, 3:4])
                        else:
                            nc.vector.tensor_scalar(hr[:], ph[:], 0.0, 0.0,
                                                    ALU.max, ALU.bypass)
                        hrs[j16] = hr

                    def zmm(j16):
                        j = j16 % 8
                        g = j16 // 8
                        nc.tensor.matmul(zt[64 * g:64 * (g + 1), :],
                                         wsig_t[:, j * 64:(j + 1) * 64],
                                         hrs[j16][:], start=(j == 0),
                                         stop=(j == 7))

                    def mask_chain(fs=fs):
                        # DVE-only chain: depends on ct/st/abc, not on z
                        q1 = work.tile([128, HF], f32, tag="q1")
                        q2 = work.tile([128, HF], f32, tag="q2")
                        q3 = work.tile([128, HF], f32, tag="q3")
                        msk = work.tile([128, HF], bf16, tag="msk")
                        nc.vector.scalar_tensor_tensor(
                            q1[:], ct_m[:, fs:fs + HF], 1.0,
                            abc_C[:, fs:fs + HF], ALU.mult, ALU.mult)
                        nc.vector.scalar_tensor_tensor(
                            q2[:], st_m[:, fs:fs + HF], 1.0,
                            abc_C[:, NC + fs:NC + fs + HF], ALU.mult, ALU.mult)
                        nc.vector.tensor_add(q3[:], q1[:], q2[:])
                        nc.vector.tensor_tensor(
                            msk[:], q3[:], abc_C[:, 2 * NC + fs:2 * NC + fs + HF],
                            ALU.is_gt)
                        return msk

                    msk = None
                    for j16 in range(LOOK):
                        hidden(j16)
                    for j16 in range(LPC):
                        if j16 + LOOK < LPC:
                            hidden(j16 + LOOK)
                        zmm(j16)
                        if j16 == 4 and pending_sum is not None:
                            pending_sum()
                            pending_sum = None
                        if last and j16 == LPC - 2:
                            # tail trim: on the final chunk the mask chain
                            # would otherwise serialize after the last zmm
                            msk = mask_chain()
                    tanhv = work.tile([128, HF], bf16, tag="tanhv")
                    nc.scalar.activation(tanhv[:], zt[:], AF.Tanh,
                                         bias=cb_t[:, 2:3], scale=0.5)
                    if msk is None:
                        # (DVE, after this hf's drains in the FIFO)
                        msk = mask_chain()
                    # vis*msk = 0.5*(tanh+1)*msk; single fused op + ONE matmul
                    tm = work.tile([128, HF], bf16, tag="tm")
                    nc.vector.scalar_tensor_tensor(tm[:], tanhv[:], 1.0, msk[:],
                                                   ALU.add, ALU.mult)

                    def make_sum(C=C, hf=hf, fs=fs, tm=tm):
                        def emit():
                            nc.tensor.matmul(
                                out_ps[:, fs:fs + HF],
                                wsum_t[:, C * 128:(C + 1) * 128], tm[:],
                                start=(C == 0), stop=(C == CHUNKS - 1))
                            if C == CHUNKS - 1:
                                out_sb = cpool.tile([128, HF], f32,
                                                    tag=f"osb{hf}")
                                nc.vector.tensor_copy(out_sb[:],
                                                      out_ps[:, fs:fs + HF])
                                nc.sync.dma_start(out[:, fs:fs + HF],
                                                  out_sb[:])
                        return emit

                    pending_sum = make_sum()

                if nxt is not None:
                    issue_mask_sins(nxt)
                    cur = nxt

            pending_sum()

    nc.compile()
    return nc


def _host_constants(points, normals, root_rot, lgtSGLobes, lgtSGLambdas,
                    W1, b1, W2, b2):
    f8 = np.float64
    lob = lgtSGLobes.astype(f8)
    l = lob / (np.linalg.norm(lob, axis=-1, keepdims=True) + TINY)
    z = np.zeros_like(l)
    z[:, 2] = 1.0
    U = np.cross(z, l)
    U = U / (np.linalg.norm(U, axis=-1, keepdims=True) + TINY)
    V = np.cross(l, U)
    V = V / (np.linalg.norm(V, axis=-1, keepdims=True) + TINY)
    sharp = lgtSGLambdas[:, 0].astype(f8)
    r_phi = np.minimum(np.arccos(1.0 - 1.0 / sharp), np.pi / 3.0)
    sp, cp = np.sin(r_phi), np.cos(r_phi)

    Wd = root_rot.astype(f8) @ W1[3:].astype(f8)          # [3,H]
    A = sp[:, None] * (U @ Wd)                             # [L,H]
    B = sp[:, None] * (V @ Wd)
    Cc = cp[:, None] * (l @ Wd)
    W1p = W1[:3].astype(f8)                                # [3,H]
    b1f = b1.astype(f8)
    w2 = W2[:, 0].astype(f8)
    w_l = np.exp(sharp * (cp - 1.0))
    scale_l = w_l / (S * w_l + TINY)
    spU = sp[:, None] * U
    spV = sp[:, None] * V
    cpl = cp[:, None] * l

    # wcst: [128, L*128]; col = l*128 + s*16 + h.  Sub-chunk layout {7,7,2}
    # within each 16-lobe chunk; per-lobe rows in its block tile:
    # ct: 8*jj+s -> -A, st: 8*m+8*jj+s -> -B, pc: 16*m..16*m+4 -> W1p,b1-C.
    wcstZ = np.zeros((128, 128, 128), f8)
    wcstV = wcstZ.reshape(128, L, 8, H)
    for ll in range(L):
        pos = ll % LPC
        k = min(pos // 7, 2)
        jj = pos - 7 * k
        m = 7 if k < 2 else 2
        for s in range(8):
            wcstV[8 * jj + s, ll, s, :] = -A[ll]
            wcstV[8 * m + 8 * jj + s, ll, s, :] = -B[ll]
        for d in range(3):
            wcstV[16 * m + d, ll, :, :] = W1p[d]
        wcstV[16 * m + 3, ll, :, :] = (b1f - Cc[ll])[None, :]

    # mask dots in [L, N] layout, f64 math rounded once to f32; c' = TINY - c
    # so the device compare stays a single is_gt.  (s-dup happens per core in
    # _make_in_maps.)
    nT = normals.astype(f8).T                              # [3, N]
    abc_a = (spU @ nT).astype(np.float32)                  # [L, N]
    abc_b = (spV @ nT).astype(np.float32)
    abc_c = (TINY - (cpl @ nT)).astype(np.float32)

    # wsig: [128, 8*64]; for in-group position p: cols p*64 + l''*8 + s' =
    # w2[h]*delta(s,s')*delta(l'',p)
    wsig = np.zeros((8, H, 8, 8, 8), f8)
    for p in range(8):
        for s in range(8):
            wsig[s, :, p, p, s] = w2
    # wsum: per-chunk [128, L] blocks with HALF the scale (tanh folding);
    # block cc maps chunk-local lobe lp to global output column cc*16+lp.
    wsum = np.zeros((LPC, 8, CHUNKS, L), f8)
    for cc in range(CHUNKS):
        for lp in range(LPC):
            wsum[lp, :, cc, cc * LPC + lp] = 0.5 * scale_l[cc * LPC + lp]

    cbias = np.zeros((128, 8), f8)
    s_of_p = np.arange(128) % 8
    # ACT Sin LUT domain is [-pi, pi]; input is r*pi/4 + bias with r in [0,1),
    # so shift each s-row by a full period where needed to stay in range.
    cos_bias = s_of_p * (np.pi / 4.0) + np.pi / 2.0 - 2.0 * np.pi * (s_of_p >= 2)
    sin_bias = s_of_p * (np.pi / 4.0) - 2.0 * np.pi * (s_of_p >= 4)
    cbias[:, 0] = cos_bias
    cbias[:, 1] = sin_bias
    cbias[:, 2] = float(b2[0]) * 0.5                      # tanh bias = b2/2
    cbias[:, 3] = 0.0                                     # relu bias
    # sub-chunk tile layouts: col5 for m=7 ([ct56|st56]), col6 for m=2
    p = np.arange(128)
    cbias[:, 5] = np.where(p < 56, cos_bias, np.where(p < 112, sin_bias, 0.0))
    cbias[:, 6] = np.where(p < 16, cos_bias, np.where(p < 32, sin_bias, 0.0))

    return dict(wcst=wcstZ.reshape(128, L * 128),
                abc_a=abc_a, abc_b=abc_b, abc_c=abc_c,
                wsig=wsig.reshape(128, 512),
                wsum=wsum.reshape(128, CHUNKS * L), cb=cbias)


def _make_in_maps(inputs):
    import ml_dtypes
    bf16 = np.dtype(ml_dtypes.bfloat16)
    f32 = np.float32

    const = _host_constants(inputs["points"], inputs["normals"],
                            inputs["root_rot"], inputs["lgtSGLobes"],
                            inputs["lgtSGLambdas"], inputs["W1"],
                            inputs["b1"], inputs["W2"], inputs["b2"])

    # replicated bf16 constant tail of `big`
    wtail = np.concatenate([const["wsig"].ravel(),
                            const["wsum"].ravel()]).astype(bf16)
    wcst = const["wcst"].astype(bf16)                      # [128, L*128]

    r_t = np.asarray(inputs["r_theta_random"], f32).transpose(1, 2, 0).reshape(L * S, N)
    pT = np.asarray(inputs["points"], f32).T
    ones = np.ones((1, N), f32)

    # block-layout theta rows (duplicated for cos|sin phases) + pc rows;
    # per chunk: [blk0(116) | blk1(116) | blk2(36 zero-padded to 116)] rows
    pc4 = np.concatenate([pT, ones], axis=0)               # [4, N]
    blocks = []
    for C in range(CHUNKS):
        ch = r_t[C * 128:(C + 1) * 128]                    # [128, N]
        for lo, m in SUBS:
            slab = ch[8 * lo:8 * (lo + m)]                 # [8m, N]
            blocks.append(slab)
            blocks.append(slab)
            blocks.append(pc4)
            if m == 2:
                blocks.append(np.zeros((80, N), f32))      # pad blk2 to 116
    inpb = np.concatenate(blocks, axis=0).astype(bf16)     # [8*348, N]

    in_maps = []
    for c in range(NCORES):
        sl = slice(c * NC, (c + 1) * NC)
        inp = np.zeros((INP_ROWS, NC), f32)
        inp[R_RT:R_RT + L * S] = r_t[:, sl]
        inp[R_CB:R_CB + 128, 0:8] = const["cb"]
        abc = np.empty((CHUNKS * 128, 3 * NC), f32)
        for C in range(CHUNKS):
            rows = slice(C * 128, (C + 1) * 128)
            ls = slice(C * LPC, (C + 1) * LPC)
            abc[rows, 0:NC] = np.repeat(const["abc_a"][ls, sl], 8, axis=0)
            abc[rows, NC:2 * NC] = np.repeat(const["abc_b"][ls, sl], 8, axis=0)
            abc[rows, 2 * NC:] = np.repeat(const["abc_c"][ls, sl], 8, axis=0)
        big = np.empty(BIG_ELEMS, bf16)
        bufv = big[O_BLK:O_WSIG].reshape(CHUNKS, 128, CB_COLS)
        core_blk = np.asarray(inpb[:, sl]).reshape(CHUNKS, 3, 116, NC)
        for C in range(CHUNKS):
            for k in range(3):
                bufv[C, 0:116, k * NC:(k + 1) * NC] = core_blk[C, k]
            bufv[C, 116:128, 0:3 * NC] = 0
            bufv[C, :, 3 * NC:] = wcst[:, C * 2048:(C + 1) * 2048]
        big[O_WSIG:] = wtail
        in_maps.append({
            "inp": np.ascontiguousarray(inp),
            "big": big,
            "abc": abc,
        })
    return in_maps


def kernel(points, normals, root_rot, lgtSGLobes, lgtSGLambdas,
           r_theta_random, W1, b1, W2, b2):
    global _PROG
    from concourse.bass_utils import run_bass_kernel_spmd

    if _PROG is None:
        _PROG = _build_program()
    nc = _PROG

    in_maps = _make_in_maps(dict(
        points=points, normals=normals, root_rot=root_rot,
        lgtSGLobes=lgtSGLobes, lgtSGLambdas=lgtSGLambdas,
        r_theta_random=r_theta_random, W1=W1, b1=b1, W2=W2, b2=b2))

    res = run_bass_kernel_spmd(nc, in_maps, list(range(NCORES)))

    f32 = np.float32
    out_full = np.empty((N, L), f32)
    for c in range(NCORES):
        out_full[c * NC:(c + 1) * NC, :] = res.results[c]["out"].T
    return out_full



# revision 38
# speedup vs baseline: 1.1353x; 1.1353x over previous
"""Trainium2 Bass kernel for the SG-visibility sampling network (v2).

Math notes (exploited structure):
  - U,V are orthogonal to the unit lobe axis l, so dot(sample_dir, l) == cos(r_phi)
    exactly.  Hence the SG weight w = exp(sharp*(cos_phi-1)) is a per-lobe
    constant and sum_s(vis*w)/(sum_s w + TINY) = scale_l * sum_s vis with
    scale_l = w/(S*w + TINY), precomputed on host.
  - pre-activation of the hidden layer decomposes as
        pre_h[n,l,s,h] = P_n[h] - C_l[h] - ct[n,l,s]*A_l[h] - st[n,l,s]*B_l[h]
    with P_n = p_n @ W1[:3] + b1,  A_l = sp_l*(U_l@Wd),  B_l = sp_l*(V_l@Wd),
    C_l = cp_l*(l_l@Wd),  Wd = root_rot @ W1[3:].
  - hemisphere mask: cos_term = ct*a_nl + st*b_nl + c_nl with
    a = normals@(sp*U)_l, b = normals@(sp*V)_l, c = normals@(cp*l)_l.
  - sigmoid(z) = 0.5*tanh(z/2) + 0.5, so vis*msk = 0.5*(tanh+1)*msk and the
    weighted sum runs as ONE accumulating matmul per (chunk, half) with
    scale_l/2 stationary.  Using Tanh instead of Sigmoid keeps every
    activation (Sin/Tanh/Relu/Copy) in ONE ACT table set (silu_and_others)
    -- zero LoadActFuncSet switches in steady state (the table-map cache is
    narrowed in _build_program so the greedy placement pass must pick it).

Device schedule (per core, data-parallel over N):
  - mask path in full fp32 (sign-exact): per-lobe a/b/c dots as 6 tiny fp32
    matmuls (TINY folded in via an ones-row so the compare is a single
    tensor_tensor is_gt), s-duplicated per chunk by one broadcast DMA each;
    mask trig = 2 fp32 Sin per chunk; q1/q2/q3/cmp + (tanh+1)*msk on DVE,
    issued AFTER each half's relu-drains so the DVE FIFO never blocks them.
  - hidden path in bf16: theta is HOST-PREBAKED into duplicated block layout
    (ct-rows | st-rows | point-rows per {7,7,2} sub-chunk) so one in-place Sin
    per sub-chunk tile produces the moving operand directly; per-lobe hidden
    matmul (bf16, K=116/36) runs LOOK=3 lobes ahead of the z matmuls (relu
    drain latency ~0.9us vs 426ns PE work per lobe), relu-drains alternate
    ACT/DVE (Pool cannot touch PSUM and walrus rejects ALU ops on Pool),
    K=128 bf16 matmul against block-diag W2 -> z in PSUM (double-buffered),
    one tanh per half, and a deferred scale matmul into the [L, n] output
    PSUM (issued mid-way through the NEXT half's z stream to dodge PE
    FIFO head-of-line blocking).
"""

import numpy as np

N, L, S, H = 8192, 128, 8, 16
NCORES = 8
NC = N // NCORES          # rays per core
LPC = 16                  # lobes per chunk
CHUNKS = L // LPC
TINY = 1e-6
HF = NC // 2              # moving-operand free-dim limit (512)

# inp (f32) row layout
R_RT = 0                  # rows 0..1023: r_theta in [l*S+s, n] layout
R_CB = L * S              # rows 1024..1151: cb [128, 8] in cols 0..7
INP_ROWS = R_CB + 128

# big (bf16, 1D): per chunk ONE [128, CB_COLS] "chunkbuf" holding the three
# theta block tiles AND the wcst stationary block side by side, so each
# partition row is a 10KB contiguous run -- DMA descriptor efficiency is
# ~4x better than per-tile 2KB rows (measured 28 -> ~113 GB/s per queue).
# cols: [blk0 | blk1 | blk2 | wcst] = [1024 | 1024 | 1024 | 2048]
SUBS = ((0, 7), (7, 7), (14, 2))   # (first lobe-in-chunk, n lobes) per sub-tile
CB_COLS = 3 * NC + LPC * 128       # 5120
O_BLK = 0
O_WSIG = O_BLK + CHUNKS * 128 * CB_COLS
O_WSUM = O_WSIG + 128 * 512
BIG_ELEMS = O_WSUM + 128 * CHUNKS * 128

_PROG = None


def _build_program():
    import concourse.bass as bass
    import concourse.bacc as bacc
    import concourse.mybir as mybir
    import concourse.tile as tile

    f32 = mybir.dt.float32
    bf16 = mybir.dt.bfloat16
    AF = mybir.ActivationFunctionType
    ALU = mybir.AluOpType
    PI4 = float(np.pi / 4.0)

    nc = bacc.Bacc("TRN2", target_bir_lowering=False, debug=False,
                   num_devices=NCORES)

    # The act-table-load pass greedily picks the FIRST table set containing
    # each activation's func, which alternates trig_and_small <-> exp_and_others
    # for Sin/Tanh (38 reloads, ~50us serialized on ACT).  silu_and_others
    # genuinely contains Sin+Tanh+Relu together; constrain the (cached) table
    # map so the pass can only pick it for Sin/Tanh.  Set ids are positional,
    # so entries are mutated in place -- never reordered.
    from concourse.hw_specs import get_activation_tables
    tabs = get_activation_tables(nc.m.arch)
    assert {AF.Sin, AF.Tanh, AF.Relu} <= tabs["silu_and_others"]
    for name, funcs in tabs.items():
        if name != "silu_and_others":
            funcs.discard(AF.Sin)
            funcs.discard(AF.Tanh)

    inp = nc.declare_dram_parameter("inp", [INP_ROWS, NC], f32, isOutput=False)
    big = nc.declare_dram_parameter("big", [BIG_ELEMS], bf16, isOutput=False)
    # host-prebaked s-duplicated hemisphere-mask dots, [128,(a|b|c')] per chunk
    abc = nc.declare_dram_parameter("abc", [CHUNKS * 128, 3 * NC], f32,
                                    isOutput=False)
    out = nc.declare_dram_parameter("out", [L, NC], f32, isOutput=True)

    def bslice(off, p, c):
        return big[off:off + p * c].rearrange("(p c) -> p c", p=p, c=c)

    # relu-drain engine per lobe-in-chunk (Pool/gpsimd cannot read PSUM, so
    # drains alternate ACT / DVE; Pool owns the SBUF-only mask chain)
    DRAIN = "ADADADADADADADAD"
    assert len(DRAIN) == LPC

    with tile.TileContext(nc) as tc:
        with (
            tc.tile_pool(name="const", bufs=1) as cpool,
            tc.tile_pool(name="io", bufs=3) as io,
            tc.tile_pool(name="trig", bufs=2) as trig,
            tc.tile_pool(name="abc", bufs=2) as abcp,
            tc.tile_pool(name="work", bufs=3) as work,
            tc.tile_pool(name="hrp", bufs=5) as hrp,
            tc.tile_pool(name="ps", bufs=4, space=bass.MemorySpace.PSUM) as ps,
            tc.tile_pool(name="zps", bufs=2, space=bass.MemorySpace.PSUM) as zps,
            tc.tile_pool(name="ops", bufs=1, space=bass.MemorySpace.PSUM) as opsp,
        ):
            # trigger the silu_and_others ACT table load during the startup
            # DMA window instead of right before the first real Sin
            warm = cpool.tile([1, 1], f32)
            nc.gpsimd.memset(warm[:], 0.0)
            nc.scalar.activation(warm[:], warm[:], AF.Sin)

            cb_t = cpool.tile([128, 8], f32)
            nc.sync.dma_start(cb_t[:], inp[R_CB:R_CB + 128, 0:8])

            out_ps = opsp.tile([128, NC], f32)
            pending_sum = None

            # One-chunk-ahead software pipeline: chunk C+1's DMAs AND all five
            # Sins (block trig + mask trig) issue at the TAIL of chunk C, so
            # chunk C's relu-drains find a conflict-free ACT FIFO and the mask
            # chain never waits on trig.
            def issue_loads(C):
                t = {}
                # one chunkbuf DMA per chunk (10KB descriptors); chunk 0
                # splits it so the blk0 Sin + first hidden matmuls can start
                # before the wcst tail lands.
                cbuf = io.tile([128, CB_COLS], bf16, tag="cbuf")
                src = big[O_BLK + C * 128 * CB_COLS:
                          O_BLK + (C + 1) * 128 * CB_COLS]
                src = src.rearrange("(p c) -> p c", p=128, c=CB_COLS)
                if C == 0:
                    # blk0 + leading wcst lobes first: gates the very first
                    # Sin + hidden matmuls (range-based dep tracking)
                    nc.sync.dma_start(cbuf[:, 0:NC], src[:, 0:NC])
                    nc.sync.dma_start(cbuf[:, 3 * NC:4 * NC], src[:, 3 * NC:4 * NC])
                    nc.sync.dma_start(cbuf[:, NC:3 * NC], src[:, NC:3 * NC])
                    nc.sync.dma_start(cbuf[:, 4 * NC:], src[:, 4 * NC:])
                else:
                    nc.sync.dma_start(cbuf[:], src)
                r_m = io.tile([128, NC], f32, tag="rm")
                nc.sync.dma_start(r_m[:], inp[C * 128:(C + 1) * 128, :])
                # host-prebaked s-dup'd a|b|c' rows, straight from HBM on the
                # scalar HWDGE queue -- issued while ACT idles waiting for the
                # first relu-drain, so neither the sync queue nor the ACT
                # compute stream pays for it.
                abc_C = abcp.tile([128, 3 * NC], f32, tag="abcC")
                nc.scalar.dma_start(abc_C[:], abc[C * 128:(C + 1) * 128, :])
                t["r_m"], t["cbuf"] = r_m, cbuf
                t["abc"] = abc_C
                return t

            def issue_block_sins(t):
                cbuf = t["cbuf"]
                nc.scalar.activation(cbuf[0:112, 0:NC], cbuf[0:112, 0:NC],
                                     AF.Sin, bias=cb_t[0:112, 5:6], scale=PI4)
                nc.scalar.activation(cbuf[0:112, NC:2 * NC],
                                     cbuf[0:112, NC:2 * NC],
                                     AF.Sin, bias=cb_t[0:112, 5:6], scale=PI4)
                nc.scalar.activation(cbuf[0:32, 2 * NC:2 * NC + NC],
                                     cbuf[0:32, 2 * NC:2 * NC + NC],
                                     AF.Sin, bias=cb_t[0:32, 6:7], scale=PI4)

            def issue_mask_sins(t):
                ct_m = trig.tile([128, NC], f32, tag="ct")
                st_m = trig.tile([128, NC], f32, tag="st")
                nc.scalar.activation(ct_m[:], t["r_m"][:], AF.Sin,
                                     bias=cb_t[:, 0:1], scale=PI4)
                nc.scalar.activation(st_m[:], t["r_m"][:], AF.Sin,
                                     bias=cb_t[:, 1:2], scale=PI4)
                t["ct"], t["st"] = ct_m, st_m

            # wsig gates the very first zmm (~3us in): small, ahead of the
            # chunk-0 loads on sync.  wsum (first sum ~14us in) rides the
            # scalar queue ahead of the bulky abc block.
            wsig_t = cpool.tile([128, 512], bf16)
            nc.sync.dma_start(wsig_t[:], bslice(O_WSIG, 128, 512))
            wsum_t = cpool.tile([128, CHUNKS * 128], bf16)
            nc.scalar.dma_start(wsum_t[:], bslice(O_WSUM, 128, CHUNKS * 128))

            cur = issue_loads(0)
            issue_block_sins(cur)
            issue_mask_sins(cur)

            for C in range(CHUNKS):
                nxt = issue_loads(C + 1) if C + 1 < CHUNKS else None
                cbuf = cur["cbuf"]
                abc_C = cur["abc"]
                ct_m, st_m = cur["ct"], cur["st"]
                last = C == CHUNKS - 1

                for hf in range(2):
                    fs = hf * HF
                    if hf == 1 and nxt is not None:
                        # chunk C+1's hidden matmuls gate on these; issuing
                        # mid-chunk keeps the ACT FIFO clear at the boundary
                        issue_block_sins(nxt)
                    zt = zps.tile([128, HF], f32, tag="zt")
                    # hidden matmuls run LOOK lobes ahead of the z matmuls so
                    # the PE FIFO never parks behind an in-flight relu-drain
                    # (drain latency ~0.9us vs 426ns of PE work per lobe).
                    LOOK = 3
                    hrs = [None] * LPC
                    # at hf1 the ACT queue starts with 3 block Sins; give the
                    # first drains to DVE there so zmm never waits on them
                    drain_eng = DRAIN if hf == 0 else DRAIN[::-1]

                    def hidden(j16):
                        k = min(j16 // 7, 2)
                        kv = 116 if k < 2 else 36
                        ph = ps.tile([128, HF], f32, tag="ph")
                        wc = 3 * NC + j16 * 128
                        nc.tensor.matmul(ph[:],
                                         cbuf[0:kv, wc:wc + 128],
                                         cbuf[0:kv, k * NC + fs:k * NC + fs + HF],
                                         start=True, stop=True)
                        hr = hrp.tile([128, HF], bf16, tag="hr")
                        if drain_eng[j16] == "A":
                            nc.scalar.activation(hr[:], ph[:], AF.Relu,
                                                 bias=cb_t[:, 3:4])
                        else:
                            nc.vector.tensor_scalar(hr[:], ph[:], 0.0, 0.0,
                                                    ALU.max, ALU.bypass)
                        hrs[j16] = hr

                    def zmm(j16):
                        j = j16 % 8
                        g = j16 // 8
                        nc.tensor.matmul(zt[64 * g:64 * (g + 1), :],
                                         wsig_t[:, j * 64:(j + 1) * 64],
                                         hrs[j16][:], start=(j == 0),
                                         stop=(j == 7))

                    def mask_chain(fs=fs):
                        # DVE-only chain: depends on ct/st/abc, not on z
                        q1 = work.tile([128, HF], f32, tag="q1")
                        q2 = work.tile([128, HF], f32, tag="q2")
                        q3 = work.tile([128, HF], f32, tag="q3")
                        msk = work.tile([128, HF], bf16, tag="msk")
                        nc.vector.scalar_tensor_tensor(
                            q1[:], ct_m[:, fs:fs + HF], 1.0,
                            abc_C[:, fs:fs + HF], ALU.mult, ALU.mult)
                        nc.vector.scalar_tensor_tensor(
                            q2[:], st_m[:, fs:fs + HF], 1.0,
                            abc_C[:, NC + fs:NC + fs + HF], ALU.mult, ALU.mult)
                        nc.vector.tensor_add(q3[:], q1[:], q2[:])
                        nc.vector.tensor_tensor(
                            msk[:], q3[:], abc_C[:, 2 * NC + fs:2 * NC + fs + HF],
                            ALU.is_gt)
                        return msk

                    msk = None
                    for j16 in range(LOOK):
                        hidden(j16)
                    for j16 in range(LPC):
                        if j16 + LOOK < LPC:
                            hidden(j16 + LOOK)
                        zmm(j16)
                        if j16 == 4 and pending_sum is not None:
                            pending_sum()
                            pending_sum = None
                        if last and j16 == LPC - 2:
                            # tail trim: on the final chunk the mask chain
                            # would otherwise serialize after the last zmm
                            msk = mask_chain()
                    tanhv = work.tile([128, HF], bf16, tag="tanhv")
                    nc.scalar.activation(tanhv[:], zt[:], AF.Tanh,
                                         bias=cb_t[:, 2:3], scale=0.5)
                    if msk is None:
                        # (DVE, after this hf's drains in the FIFO)
                        msk = mask_chain()
                    # vis*msk = 0.5*(tanh+1)*msk; single fused op + ONE matmul
                    tm = work.tile([128, HF], bf16, tag="tm")
                    nc.vector.scalar_tensor_tensor(tm[:], tanhv[:], 1.0, msk[:],
                                                   ALU.add, ALU.mult)

                    def make_sum(C=C, hf=hf, fs=fs, tm=tm):
                        def emit():
                            nc.tensor.matmul(
                                out_ps[:, fs:fs + HF],
                                wsum_t[:, C * 128:(C + 1) * 128], tm[:],
                                start=(C == 0), stop=(C == CHUNKS - 1))
                            if C == CHUNKS - 1:
                                out_sb = cpool.tile([128, HF], f32,
                                                    tag=f"osb{hf}")
                                nc.vector.tensor_copy(out_sb[:],
                                                      out_ps[:, fs:fs + HF])
                                nc.sync.dma_start(out[:, fs:fs + HF],
                                                  out_sb[:])
                        return emit

                    pending_sum = make_sum()

                if nxt is not None:
                    issue_mask_sins(nxt)
                    cur = nxt

            pending_sum()

    nc.compile()
    return nc


def _host_constants(points, normals, root_rot, lgtSGLobes, lgtSGLambdas,
                    W1, b1, W2, b2):
    f8 = np.float64
    lob = lgtSGLobes.astype(f8)
    l = lob / (np.linalg.norm(lob, axis=-1, keepdims=True) + TINY)
    z = np.zeros_like(l)
    z[:, 2] = 1.0
    U = np.cross(z, l)
    U = U / (np.linalg.norm(U, axis=-1, keepdims=True) + TINY)
    V = np.cross(l, U)
    V = V / (np.linalg.norm(V, axis=-1, keepdims=True) + TINY)
    sharp = lgtSGLambdas[:, 0].astype(f8)
    r_phi = np.minimum(np.arccos(1.0 - 1.0 / sharp), np.pi / 3.0)
    sp, cp = np.sin(r_phi), np.cos(r_phi)

    Wd = root_rot.astype(f8) @ W1[3:].astype(f8)          # [3,H]
    A = sp[:, None] * (U @ Wd)                             # [L,H]
    B = sp[:, None] * (V @ Wd)
    Cc = cp[:, None] * (l @ Wd)
    W1p = W1[:3].astype(f8)                                # [3,H]
    b1f = b1.astype(f8)
    w2 = W2[:, 0].astype(f8)
    w_l = np.exp(sharp * (cp - 1.0))
    scale_l = w_l / (S * w_l + TINY)
    spU = sp[:, None] * U
    spV = sp[:, None] * V
    cpl = cp[:, None] * l

    # wcst: [128, L*128]; col = l*128 + s*16 + h.  Sub-chunk layout {7,7,2}
    # within each 16-lobe chunk; per-lobe rows in its block tile:
    # ct: 8*jj+s -> -A, st: 8*m+8*jj+s -> -B, pc: 16*m..16*m+4 -> W1p,b1-C.
    wcstZ = np.zeros((128, 128, 128), f8)
    wcstV = wcstZ.reshape(128, L, 8, H)
    for ll in range(L):
        pos = ll % LPC
        k = min(pos // 7, 2)
        jj = pos - 7 * k
        m = 7 if k < 2 else 2
        for s in range(8):
            wcstV[8 * jj + s, ll, s, :] = -A[ll]
            wcstV[8 * m + 8 * jj + s, ll, s, :] = -B[ll]
        for d in range(3):
            wcstV[16 * m + d, ll, :, :] = W1p[d]
        wcstV[16 * m + 3, ll, :, :] = (b1f - Cc[ll])[None, :]

    # mask dots in [L, N] layout, f64 math rounded once to f32; c' = TINY - c
    # so the device compare stays a single is_gt.  (s-dup happens per core in
    # _make_in_maps.)
    nT = normals.astype(f8).T                              # [3, N]
    abc_a = (spU @ nT).astype(np.float32)                  # [L, N]
    abc_b = (spV @ nT).astype(np.float32)
    abc_c = (TINY - (cpl @ nT)).astype(np.float32)

    # wsig: [128, 8*64]; for in-group position p: cols p*64 + l''*8 + s' =
    # w2[h]*delta(s,s')*delta(l'',p)
    wsig = np.zeros((8, H, 8, 8, 8), f8)
    for p in range(8):
        for s in range(8):
            wsig[s, :, p, p, s] = w2
    # wsum: per-chunk [128, L] blocks with HALF the scale (tanh folding);
    # block cc maps chunk-local lobe lp to global output column cc*16+lp.
    wsum = np.zeros((LPC, 8, CHUNKS, L), f8)
    for cc in range(CHUNKS):
        for lp in range(LPC):
            wsum[lp, :, cc, cc * LPC + lp] = 0.5 * scale_l[cc * LPC + lp]

    cbias = np.zeros((128, 8), f8)
    s_of_p = np.arange(128) % 8
    # ACT Sin LUT domain is [-pi, pi]; input is r*pi/4 + bias with r in [0,1),
    # so shift each s-row by a full period where needed to stay in range.
    cos_bias = s_of_p * (np.pi / 4.0) + np.pi / 2.0 - 2.0 * np.pi * (s_of_p >= 2)
    sin_bias = s_of_p * (np.pi / 4.0) - 2.0 * np.pi * (s_of_p >= 4)
    cbias[:, 0] = cos_bias
    cbias[:, 1] = sin_bias
    cbias[:, 2] = float(b2[0]) * 0.5                      # tanh bias = b2/2
    cbias[:, 3] = 0.0                                     # relu bias
    # sub-chunk tile layouts: col5 for m=7 ([ct56|st56]), col6 for m=2
    p = np.arange(128)
    cbias[:, 5] = np.where(p < 56, cos_bias, np.where(p < 112, sin_bias, 0.0))
    cbias[:, 6] = np.where(p < 16, cos_bias, np.where(p < 32, sin_bias, 0.0))

    return dict(wcst=wcstZ.reshape(128, L * 128),
                abc_a=abc_a, abc_b=abc_b, abc_c=abc_c,
                wsig=wsig.reshape(128, 512),
                wsum=wsum.reshape(128, CHUNKS * L), cb=cbias)


def _make_in_maps(inputs):
    import ml_dtypes
    bf16 = np.dtype(ml_dtypes.bfloat16)
    f32 = np.float32

    const = _host_constants(inputs["points"], inputs["normals"],
                            inputs["root_rot"], inputs["lgtSGLobes"],
                            inputs["lgtSGLambdas"], inputs["W1"],
                            inputs["b1"], inputs["W2"], inputs["b2"])

    # replicated bf16 constant tail of `big`
    wtail = np.concatenate([const["wsig"].ravel(),
                            const["wsum"].ravel()]).astype(bf16)
    wcst = const["wcst"].astype(bf16)                      # [128, L*128]

    r_t = np.asarray(inputs["r_theta_random"], f32).transpose(1, 2, 0).reshape(L * S, N)
    pT = np.asarray(inputs["points"], f32).T
    ones = np.ones((1, N), f32)

    # block-layout theta rows (duplicated for cos|sin phases) + pc rows;
    # per chunk: [blk0(116) | blk1(116) | blk2(36 zero-padded to 116)] rows
    pc4 = np.concatenate([pT, ones], axis=0)               # [4, N]
    blocks = []
    for C in range(CHUNKS):
        ch = r_t[C * 128:(C + 1) * 128]                    # [128, N]
        for lo, m in SUBS:
            slab = ch[8 * lo:8 * (lo + m)]                 # [8m, N]
            blocks.append(slab)
            blocks.append(slab)
            blocks.append(pc4)
            if m == 2:
                blocks.append(np.zeros((80, N), f32))      # pad blk2 to 116
    inpb = np.concatenate(blocks, axis=0).astype(bf16)     # [8*348, N]

    in_maps = []
    for c in range(NCORES):
        sl = slice(c * NC, (c + 1) * NC)
        inp = np.zeros((INP_ROWS, NC), f32)
        inp[R_RT:R_RT + L * S] = r_t[:, sl]
        inp[R_CB:R_CB + 128, 0:8] = const["cb"]
        abc = np.empty((CHUNKS * 128, 3 * NC), f32)
        for C in range(CHUNKS):
            rows = slice(C * 128, (C + 1) * 128)
            ls = slice(C * LPC, (C + 1) * LPC)
            abc[rows, 0:NC] = np.repeat(const["abc_a"][ls, sl], 8, axis=0)
            abc[rows, NC:2 * NC] = np.repeat(const["abc_b"][ls, sl], 8, axis=0)
            abc[rows, 2 * NC:] = np.repeat(const["abc_c"][ls, sl], 8, axis=0)
        big = np.empty(BIG_ELEMS, bf16)
        bufv = big[O_BLK:O_WSIG].reshape(CHUNKS, 128, CB_COLS)
        core_blk = np.asarray(inpb[:, sl]).reshape(CHUNKS, 3, 116, NC)
        for C in range(CHUNKS):
            for k in range(3):
                bufv[C, 0:116, k * NC:(k + 1) * NC] = core_blk[C, k]
            bufv[C, 116:128, 0:3 * NC] = 0
            bufv[C, :, 3 * NC:] = wcst[:, C * 2048:(C + 1) * 2048]
        big[O_WSIG:] = wtail
        in_maps.append({
            "inp": np.ascontiguousarray(inp),
            "big": big,
            "abc": abc,
        })
    return in_maps


def kernel(points, normals, root_rot, lgtSGLobes, lgtSGLambdas,
           r_theta_random, W1, b1, W2, b2):
    global _PROG
    from concourse.bass_utils import run_bass_kernel_spmd

    if _PROG is None:
        _PROG = _build_program()
    nc = _PROG

    in_maps = _make_in_maps(dict(
        points=points, normals=normals, root_rot=root_rot,
        lgtSGLobes=lgtSGLobes, lgtSGLambdas=lgtSGLambdas,
        r_theta_random=r_theta_random, W1=W1, b1=b1, W2=W2, b2=b2))

    res = run_bass_kernel_spmd(nc, in_maps, list(range(NCORES)))

    f32 = np.float32
    out_full = np.empty((N, L), f32)
    for c in range(NCORES):
        out_full[c * NC:(c + 1) * NC, :] = res.results[c]["out"].T
    return out_full

